# revision 38
# baseline (speedup 1.0000x reference)
"""Bass/Trainium2 kernel for batched kNN-interpolate + MSE (nn_KnnMSE).

Reference computation:
  d2[i,j] = ||c2_i - c1_j||^2, masked to same-graph pairs (b1/b2 sorted),
  top-k=8 smallest per target row, w = 1/clip(d2, 1e-16),
  interp = sum(w * f1[idx]) / sum(w),  out = mean((interp - f2)^2).

b1/b2 are SORTED graph ids => the distance matrix is block-diagonal over
the 64 graphs.  SPMD over 8 cores requires one program, so the 64 graphs
are sorted by target count and dealt into 8 rank-groups ("slots"); slot i
of every core gets one graph from rank-group i.  Per-slot shapes are the
group maxima, which track the individual graph sizes closely (vs the old
fixed 320/384 padding):  chunk counts nt=ceil(n2/128) (mostly 2-3),
source width s = max n1 rounded up (~260-300 vs 320).

Per 128-target chunk of a slot:
  1. PE : m = -d2 = 2*c2.c1 - ||c1||^2 - ||c2||^2  (fp16 hi/lo rows, K=13;
          both norms folded into the matmul so no bias is needed anywhere)
  2. ACT: rec = Reciprocal(-m) = 1/d2  (raw InstActivation -- the bass
          wrapper's accuracy guard is irrelevant at our 2e-2 tolerance;
          measured 1.2e-5 rel err on hardware) -> fp16 SBUF
  3. DVE: top8 = max8(rec); th = top8[:,7]
  4. DVE: W = (rec >= th) * rec  (fp16 2x mode), sumw = rowsum  [fused stt]
  5. DVE: rsw = 1/sumw  (one [128, nt] reciprocal per slot)
  6. GPS: diag[t] = ident * rsw[:,t]  (fp16 diagonal matrix, gpsimd)
  7. PE : Wt[t] = W[t]^T @ diag[t]  (transpose + 1/sumw normalization fused
          into one regular matmul: lhsT=W chunk, rhs=diag)
  8. ACT: wts = one batched PSUM->SBUF fp16 copy per target chunk
  9. PE : interp = sum_k wts[k]^T @ f1a[k]  (PSUM accum; already normalized)
 10. DVE: err = interp - f2  (one tensor_tensor per slot, [128, nt*128])
 11. ACT/DVE alternating: acc[:,slot] = sum(err^2)  (one fused op per slot)
 finally: acc -> rowsum -> DRAM; host sums 8x128 values / (N*D).

Padding: pad sources at coords (BIGC,)*3, pad targets at (BIGC+1,)*3.
Pads are ~3e4 away from all real points (never selected by real targets,
never select real neighbors with meaningful weight), while pad targets
land within sqrt(3) of the pad sources whose features are 0 => their
interp ~ 1e-3 and err^2 ~ 1e-6 * f2(=0): contributes < 1e-5 relative to
the MSE, so no validity masking is needed anywhere.  s >= max(n1)+1
guarantees at least one pad source per slot.
"""

import numpy as np

# Problem constants
N = 16384
D = 128
B = 64
KNN = 8
NCORES = 8
GPC = B // NCORES        # graphs (slots) per core
BIGC = 100.0             # padded-source coordinate
KMM = 13                 # dist rows: 9 coord hi/lo cross terms + 2+2 norm rows

_NC_CACHE = {}


def _build_nc(slot_sizes):
    """slot_sizes: tuple of GPC tuples (s, nt, ns)."""
    import concourse.bacc as bacc
    import concourse.mybir as mybir
    import concourse.tile as tile
    from concourse.masks import make_identity

    f32 = mybir.dt.float32
    f16 = mybir.dt.float16
    AF = mybir.ActivationFunctionType
    OP = mybir.AluOpType

    smax = max(s for s, _, _ in slot_sizes)
    ntmax = max(nt for _, nt, _ in slot_sizes)
    nsmax = max(ns for _, _, ns in slot_sizes)
    SS = sum(s for s, _, _ in slot_sizes)
    NTC = sum(nt for _, nt, _ in slot_sizes)
    NSC = sum(ns for _, _, ns in slot_sizes)

    nc = bacc.Bacc("TRN2", target_bir_lowering=False, debug=False)

    c1r_d = nc.dram_tensor("c1r", [KMM, SS], f16, kind="ExternalInput")
    c2t_d = nc.dram_tensor("c2t", [KMM, NTC * 128], f16, kind="ExternalInput")
    f1a_d = nc.dram_tensor("f1a", [128, NSC, D], f16, kind="ExternalInput")
    f2_d = nc.dram_tensor("f2", [128, NTC, D], f16, kind="ExternalInput")
    out_d = nc.dram_tensor("out_sums", [128, 1], f32, kind="ExternalOutput")

    def recip_raw(out, in_):
        """out = 1/(-in_): nc.scalar.activation(Reciprocal, scale=-1) minus
        the wrapper's accuracy guard (fine at our tolerance)."""
        eng = nc.scalar
        ins = [eng.lower_ap(in_)]
        for arg in (0.0, -1.0, 0.0):  # bias, scale, alpha
            ins.append(mybir.ImmediateValue(dtype=mybir.dt.float32, value=arg))
        return eng.add_instruction(
            mybir.InstActivation(
                name=eng.bass.get_next_instruction_name(),
                func=AF.Reciprocal,
                ins=ins,
                outs=[eng.lower_ap(out)],
            )
        )

    with tile.TileContext(nc) as tc:
        with (
            tc.tile_pool(name="constp", bufs=1) as constp,
            tc.tile_pool(name="recp", bufs=8) as recp,
            tc.tile_pool(name="wp", bufs=8) as wp,
            tc.tile_pool(name="wtsp", bufs=5) as wtsp,
            tc.tile_pool(name="diagp", bufs=6) as diagp,
            tc.tile_pool(name="smallp", bufs=8) as smallp,
            tc.tile_pool(name="errp", bufs=6) as errp,
            tc.tile_pool(name="pup", bufs=3, space="PSUM") as pup,
            tc.tile_pool(name="pwtp", bufs=3, space="PSUM") as pwtp,
            tc.tile_pool(name="pip", bufs=2, space="PSUM") as pip_,
        ):
            ident = constp.tile([128, 128], f16)
            make_identity(nc, ident)
            acc = constp.tile([128, GPC], f32)
            nc.vector.memset(acc, 0.0)

            # inputs: few big DMAs (sync + gpsimd queues)
            c1r_t = constp.tile([KMM, SS], f16)
            nc.sync.dma_start(c1r_t, c1r_d[:, :])
            c2t_t = constp.tile([KMM, NTC * 128], f16)
            nc.sync.dma_start(c2t_t, c2t_d[:, :])
            f1a_t = constp.tile([128, NSC, D], f16)
            h = NSC // 2
            nc.sync.dma_start(f1a_t[:, :h], f1a_d[:, :h])
            nc.sync.dma_start(f1a_t[:, h:], f1a_d[:, h:])
            f2_t = constp.tile([128, NTC, D], f16)
            h2 = NTC // 2
            nc.gpsimd.dma_start(f2_t[:, :h2], f2_d[:, :h2])
            nc.gpsimd.dma_start(f2_t[:, h2:], f2_d[:, h2:])

            # per-slot offsets
            soffs, ntoffs, nsoffs = [], [], []
            so = nto = nso = 0
            for s_g, nt, ns in slot_sizes:
                soffs.append(so)
                ntoffs.append(nto)
                nsoffs.append(nso)
                so += s_g
                nto += nt
                nso += ns

            def emit_dist(g):
                """PE: m = -d2 [128, s_g] per target chunk of slot g."""
                s_g, nt, _ = slot_sizes[g]
                pus = []
                for t in range(nt):
                    pu = pup.tile([128, smax], f32, tag="pu")
                    nc.tensor.matmul(
                        pu[:, :s_g],
                        c2t_t[:, (ntoffs[g] + t) * 128 : (ntoffs[g] + t + 1) * 128],
                        c1r_t[:, soffs[g] : soffs[g] + s_g],
                        start=True,
                        stop=True,
                    )
                    pus.append(pu)
                return pus

            for g, (s_g, nt, ns) in enumerate(slot_sizes):
                soff, ntoff, nsoff = soffs[g], ntoffs[g], nsoffs[g]
                pus = emit_dist(g)
                # 2) rec = 1/d2
                recs = []
                for t in range(nt):
                    rec = recp.tile([128, smax], f16, tag="rec")
                    recip_raw(rec[:, :s_g], pus[t][:, :s_g])
                    recs.append(rec)
                # 3) top8
                top8s = []
                for t in range(nt):
                    top8 = smallp.tile([128, 8], f16, tag="top8")
                    nc.vector.max(out=top8, in_=recs[t][:, :s_g])
                    top8s.append(top8)
                # 4) W = (rec >= th) * rec, sumw = rowsum(W)
                sumw = smallp.tile([128, ntmax], f32, tag="sumw")
                Ws = []
                for t in range(nt):
                    W = wp.tile([128, smax], f16, tag="W")
                    nc.vector.scalar_tensor_tensor(
                        out=W[:, :s_g],
                        in0=recs[t][:, :s_g],
                        scalar=top8s[t][:, 7:8],
                        in1=recs[t][:, :s_g],
                        op0=OP.is_ge,
                        op1=OP.mult,
                        accum_out=sumw[:, t : t + 1],
                    )
                    Ws.append(W)
                # 5) rsw = 1/sumw
                rsw = smallp.tile([128, ntmax], f32, tag="rsw")
                nc.vector.reciprocal(rsw[:, :nt], sumw[:, :nt])
                # 6) diag[t] = ident * rsw[:,t] -- all nt diagonals in ONE
                #    broadcast tensor_tensor: rsw bcast along j, ident along t
                dgc = diagp.tile([128, ntmax, 128], f16, tag="diag")
                nc.vector.tensor_tensor(
                    out=dgc[:, :nt],
                    in0=rsw[:, :nt].unsqueeze(2).broadcast_to([128, nt, 128]),
                    in1=ident.unsqueeze(1).broadcast_to([128, nt, 128]),
                    op=OP.mult,
                )
                diags = [dgc[:, t] for t in range(nt)]
                # 7) Wt[t] = W[t]^T @ diag[t]  (transpose + normalize fused)
                pwts = []
                for t in range(nt):
                    pwt = pwtp.tile([128, nsmax, 128], f32, tag="pwt")
                    for k in range(ns):
                        w0 = 128 * k
                        cw = min(s_g, w0 + 128) - w0
                        nc.tensor.matmul(
                            pwt[:cw, k],
                            Ws[t][:, w0 : w0 + cw],
                            diags[t],
                            start=True,
                            stop=True,
                        )
                    pwts.append(pwt)
                # 8) one batched PSUM->SBUF copy per target chunk
                wtss = []
                for t in range(nt):
                    wts = wtsp.tile([128, nsmax, 128], f16, tag="wts")
                    nc.scalar.copy(wts[:, :ns], pwts[t][:, :ns])
                    wtss.append(wts)
                # 9) interp[t] = sum_k wts[k]^T @ f1a[k]
                pi = pip_.tile([128, ntmax, D], f32, tag="pi")
                for t in range(nt):
                    for k in range(ns):
                        w0 = 128 * k
                        cw = min(s_g, w0 + 128) - w0
                        nc.tensor.matmul(
                            pi[:, t],
                            wtss[t][:cw, k],
                            f1a_t[:cw, nsoff + k],
                            start=(k == 0),
                            stop=(k == ns - 1),
                        )
                # 10) err = interp - f2  (one op per slot)
                err = errp.tile([128, ntmax, D], f16, tag="err")
                nc.vector.tensor_tensor(
                    out=err[:, :nt],
                    in0=pi[:, :nt],
                    in1=f2_t[:, ntoff : ntoff + nt],
                    op=OP.subtract,
                )
                # 11) acc[:, g] = sum(err^2) on ACT (DVE is the critical
                #     engine in steady state)
                sq = errp.tile([128, ntmax, D], f16, tag="sq")
                nc.scalar.activation(
                    sq[:, :nt],
                    err[:, :nt],
                    AF.Square,
                    accum_out=acc[:, g : g + 1],
                )

            tot = constp.tile([128, 1], f32)
            nc.vector.reduce_sum(tot, acc, axis=mybir.AxisListType.X)
            nc.sync.dma_start(out_d[:, :], tot)

    nc.compile()
    return nc


def _hl(x):
    """fp16 hi/lo split: x ~= hi + lo with both parts exact in fp16."""
    hi = x.astype(np.float16)
    lo = (x - hi.astype(np.float32)).astype(np.float16)
    return hi, lo


def _prep(inputs):
    """Returns (slot_sizes, in_maps): slot-shaped packing of the 64 graphs."""
    x1 = np.ascontiguousarray(np.asarray(inputs["x1"], dtype=np.float32))
    x2 = np.ascontiguousarray(np.asarray(inputs["x2"], dtype=np.float32))
    b1 = np.asarray(inputs["b1"]).astype(np.int64)
    b2 = np.asarray(inputs["b2"]).astype(np.int64)

    c1, f1 = x1[:, :3], x1[:, 3:]
    c2, f2 = x2[:, :3], x2[:, 3:]

    gs = np.arange(B + 1)
    e1 = np.searchsorted(b1, gs)
    e2 = np.searchsorted(b2, gs)
    n1 = np.diff(e1)
    n2 = np.diff(e2)
    assert n1.min() >= KNN, f"graph with fewer than {KNN} sources"
    assert n2.min() >= 1

    # Deal graphs into GPC rank-groups; group i -> slot i of every core.
    # Group by nt-class (ceil(n2/128)) so per-slot nt tracks each graph's
    # own chunk count, then WITHIN each nt-class group by n1 rank so that
    # per-slot source width (group max n1) is as tight as possible -- this
    # gives about half the slots ns=2 instead of all ns=3.
    order = np.argsort(-n2, kind="stable")
    nt_of = -(-n2 // 128)
    classes = {}
    for g in order:
        classes.setdefault(int(nt_of[g]), []).append(int(g))
    groups = []
    leftover = []
    for ntc in sorted(classes, reverse=True):
        glist = classes[ntc]
        glist = sorted(glist, key=lambda g: -int(n1[g]))
        while len(glist) >= NCORES:
            groups.append(np.array(glist[:NCORES]))
            glist = glist[NCORES:]
        leftover.extend(glist)
    while leftover:
        groups.append(np.array(leftover[:NCORES]))
        leftover = leftover[NCORES:]
    assert len(groups) == GPC and all(len(g) == NCORES for g in groups)

    slot_sizes = []
    for grp in groups:
        s = max(int(n1[grp].max()) + 1, KNN + 1)
        s = -(-s // 4) * 4
        nt = -(-int(n2[grp].max()) // 128)
        ns = -(-s // 128)
        slot_sizes.append((s, nt, ns))
    slot_sizes = tuple(slot_sizes)

    SS = sum(s for s, _, _ in slot_sizes)
    NTC = sum(nt for _, nt, _ in slot_sizes)
    NSC = sum(ns for _, _, ns in slot_sizes)

    in_maps = []
    for c in range(NCORES):
        c1r = np.zeros((KMM, SS), np.float16)
        c2t = np.zeros((KMM, NTC * 128), np.float16)
        f1a = np.zeros((128, NSC, D), np.float16)
        f2p = np.zeros((128, NTC, D), np.float16)

        soff = ntoff = nsoff = 0
        for gi in range(GPC):
            g = int(groups[gi][c])
            s_g, nt, ns = slot_sizes[gi]
            a, bnd = e1[g], e1[g + 1]
            n = int(n1[g])
            cc = np.full((s_g, 3), BIGC, np.float32)
            cc[:n] = c1[a:bnd]
            h1, l1 = _hl(cc)
            nrm1 = np.einsum("ij,ij->i", cc, cc)
            n1h, n1l = _hl(nrm1)
            blk = c1r[:, soff : soff + s_g]
            blk[0:3] = (2.0 * h1.astype(np.float32)).astype(np.float16).T
            blk[3:6] = (2.0 * l1.astype(np.float32)).astype(np.float16).T
            blk[6:9] = blk[0:3]
            blk[9] = -n1h
            blk[10] = -n1l
            blk[11:13] = 1.0

            ff = np.zeros((ns * 128, D), np.float32)
            ff[:n] = f1[a:bnd]
            f1a[:, nsoff : nsoff + ns] = (
                ff.reshape(ns, 128, D).transpose(1, 0, 2).astype(np.float16)
            )

            a2, bnd2 = e2[g], e2[g + 1]
            m = int(n2[g])
            tcd = np.full((nt * 128, 3), BIGC + 1.0, np.float32)
            tcd[:m] = c2[a2:bnd2]
            h2, l2 = _hl(tcd)
            nrm2 = np.einsum("ij,ij->i", tcd, tcd)
            n2h, n2l = _hl(nrm2)
            tblk = c2t[:, ntoff * 128 : (ntoff + nt) * 128]
            tblk[0:3] = h2.T
            tblk[3:6] = h2.T
            tblk[6:9] = l2.T
            tblk[9:11] = 1.0
            tblk[11] = -n2h
            tblk[12] = -n2l

            fv = np.zeros((nt * 128, D), np.float32)
            fv[:m] = f2[a2:bnd2]
            f2p[:, ntoff : ntoff + nt] = (
                fv.reshape(nt, 128, D).transpose(1, 0, 2).astype(np.float16)
            )

            soff += s_g
            ntoff += nt
            nsoff += ns

        in_maps.append(
            {
                "c1r": np.ascontiguousarray(c1r),
                "c2t": np.ascontiguousarray(c2t),
                "f1a": np.ascontiguousarray(f1a),
                "f2": np.ascontiguousarray(f2p),
            }
        )
    return slot_sizes, in_maps


def run(inputs, trace=False):
    """Returns (mse_scalar_f32, exec_time_ns_or_None)."""
    from concourse.bass_utils import run_bass_kernel_spmd

    slot_sizes, in_maps = _prep(inputs)
    nc = _NC_CACHE.get(slot_sizes)
    if nc is None:
        nc = _NC_CACHE.setdefault(slot_sizes, _build_nc(slot_sizes))
    res = run_bass_kernel_spmd(nc, in_maps, core_ids=list(range(NCORES)), trace=trace)
    total = 0.0
    for r in res.results:
        total += np.asarray(r["out_sums"], dtype=np.float64).sum()
    mse = np.float32(total / (N * D))
    return mse, res.exec_time_ns


def kernel(**inputs):
    out, _ = run(inputs, trace=False)
    return out


# revision 39
# speedup vs baseline: 1.0339x; 1.0339x over previous
"""Bass/Trainium2 kernel for batched kNN-interpolate + MSE (nn_KnnMSE).

Reference computation:
  d2[i,j] = ||c2_i - c1_j||^2, masked to same-graph pairs (b1/b2 sorted),
  top-k=8 smallest per target row, w = 1/clip(d2, 1e-16),
  interp = sum(w * f1[idx]) / sum(w),  out = mean((interp - f2)^2).

b1/b2 are SORTED graph ids => the distance matrix is block-diagonal over
the 64 graphs.  SPMD over 8 cores requires one program, so the 64 graphs
are sorted by target count and dealt into 8 rank-groups ("slots"); slot i
of every core gets one graph from rank-group i.  Per-slot shapes are the
group maxima, which track the individual graph sizes closely (vs the old
fixed 320/384 padding):  chunk counts nt=ceil(n2/128) (mostly 2-3),
source width s = max n1 rounded up (~260-300 vs 320).

Per 128-target chunk of a slot:
  1. PE : m = -d2 = 2*c2.c1 - ||c1||^2 - ||c2||^2  (fp16 hi/lo rows, K=13;
          both norms folded into the matmul so no bias is needed anywhere)
  2. ACT: rec = Reciprocal(-m) = 1/d2  (raw InstActivation -- the bass
          wrapper's accuracy guard is irrelevant at our 2e-2 tolerance;
          measured 1.2e-5 rel err on hardware) -> fp16 SBUF
  3. DVE: top8 = max8(rec); th = top8[:,7]
  4. DVE: W = (rec >= th) * rec  (fp16 2x mode), sumw = rowsum  [fused stt]
  5. DVE: rsw = 1/sumw  (one [128, nt] reciprocal per slot)
  6. GPS: diag[t] = ident * rsw[:,t]  (fp16 diagonal matrix, gpsimd)
  7. PE : Wt[t] = W[t]^T @ diag[t]  (transpose + 1/sumw normalization fused
          into one regular matmul: lhsT=W chunk, rhs=diag)
  8. ACT: wts = one batched PSUM->SBUF fp16 copy per target chunk
  9. PE : interp = sum_k wts[k]^T @ f1a[k]  (PSUM accum; already normalized)
 10. DVE: err = interp - f2  (one tensor_tensor per slot, [128, nt*128])
 11. ACT/DVE alternating: acc[:,slot] = sum(err^2)  (one fused op per slot)
 finally: acc -> rowsum -> DRAM; host sums 8x128 values / (N*D).

Padding: pad sources at coords (BIGC,)*3, pad targets at (BIGC+1,)*3.
Pads are ~3e4 away from all real points (never selected by real targets,
never select real neighbors with meaningful weight), while pad targets
land within sqrt(3) of the pad sources whose features are 0 => their
interp ~ 1e-3 and err^2 ~ 1e-6 * f2(=0): contributes < 1e-5 relative to
the MSE, so no validity masking is needed anywhere.  s >= max(n1)+1
guarantees at least one pad source per slot.
"""

import numpy as np

# Problem constants
N = 16384
D = 128
B = 64
KNN = 8
NCORES = 8
GPC = B // NCORES        # graphs (slots) per core
BIGC = 100.0             # padded-source coordinate
KMM = 13                 # dist rows: 9 coord hi/lo cross terms + 2+2 norm rows

_NC_CACHE = {}


def _build_nc(slot_sizes):
    """slot_sizes: tuple of GPC tuples (s, nt, ns)."""
    import concourse.bacc as bacc
    import concourse.mybir as mybir
    import concourse.tile as tile
    from concourse.masks import make_identity

    f32 = mybir.dt.float32
    f16 = mybir.dt.float16
    AF = mybir.ActivationFunctionType
    OP = mybir.AluOpType

    smax = max(s for s, _, _ in slot_sizes)
    ntmax = max(nt for _, nt, _ in slot_sizes)
    nsmax = max(ns for _, _, ns in slot_sizes)
    SS = sum(s for s, _, _ in slot_sizes)
    NTC = sum(nt for _, nt, _ in slot_sizes)
    NSC = sum(ns for _, _, ns in slot_sizes)

    nc = bacc.Bacc("TRN2", target_bir_lowering=False, debug=False)

    c1r_d = nc.dram_tensor("c1r", [KMM, SS], f16, kind="ExternalInput")
    c2t_d = nc.dram_tensor("c2t", [KMM, NTC * 128], f16, kind="ExternalInput")
    f1a_d = nc.dram_tensor("f1a", [128, NSC, D], f16, kind="ExternalInput")
    f2_d = nc.dram_tensor("f2", [128, NTC, D], f16, kind="ExternalInput")
    out_d = nc.dram_tensor("out_sums", [128, 1], f32, kind="ExternalOutput")

    def recip_raw(out, in_):
        """out = 1/(-in_): nc.scalar.activation(Reciprocal, scale=-1) minus
        the wrapper's accuracy guard (fine at our tolerance)."""
        eng = nc.scalar
        ins = [eng.lower_ap(in_)]
        for arg in (0.0, -1.0, 0.0):  # bias, scale, alpha
            ins.append(mybir.ImmediateValue(dtype=mybir.dt.float32, value=arg))
        return eng.add_instruction(
            mybir.InstActivation(
                name=eng.bass.get_next_instruction_name(),
                func=AF.Reciprocal,
                ins=ins,
                outs=[eng.lower_ap(out)],
            )
        )

    with tile.TileContext(nc) as tc:
        with (
            tc.tile_pool(name="constp", bufs=1) as constp,
            tc.tile_pool(name="recp", bufs=6) as recp,
            tc.tile_pool(name="wp", bufs=6) as wp,
            tc.tile_pool(name="wtsp", bufs=5) as wtsp,
            tc.tile_pool(name="diagp", bufs=6) as diagp,
            tc.tile_pool(name="smallp", bufs=8) as smallp,
            tc.tile_pool(name="errp", bufs=4) as errp,
            tc.tile_pool(name="pup", bufs=3, space="PSUM") as pup,
            tc.tile_pool(name="pwtp", bufs=3, space="PSUM") as pwtp,
            tc.tile_pool(name="pip", bufs=2, space="PSUM") as pip_,
        ):
            ident = constp.tile([128, 128], f16)
            make_identity(nc, ident)
            acc = constp.tile([128, GPC], f32)
            nc.vector.memset(acc, 0.0)

            # inputs: few big DMAs (sync + gpsimd queues)
            c1r_t = constp.tile([KMM, SS], f16)
            nc.sync.dma_start(c1r_t, c1r_d[:, :])
            c2t_t = constp.tile([KMM, NTC * 128], f16)
            nc.sync.dma_start(c2t_t, c2t_d[:, :])
            f1a_t = constp.tile([128, NSC, D], f16)
            h = NSC // 2
            nc.sync.dma_start(f1a_t[:, :h], f1a_d[:, :h])
            nc.sync.dma_start(f1a_t[:, h:], f1a_d[:, h:])
            f2_t = constp.tile([128, NTC, D], f16)
            h2 = NTC // 2
            nc.gpsimd.dma_start(f2_t[:, :h2], f2_d[:, :h2])
            nc.gpsimd.dma_start(f2_t[:, h2:], f2_d[:, h2:])

            # per-slot offsets
            soffs, ntoffs, nsoffs = [], [], []
            so = nto = nso = 0
            for s_g, nt, ns in slot_sizes:
                soffs.append(so)
                ntoffs.append(nto)
                nsoffs.append(nso)
                so += s_g
                nto += nt
                nso += ns

            def emit_dist(g):
                """PE: m = -d2 [128, s_g] per target chunk of slot g."""
                s_g, nt, _ = slot_sizes[g]
                pus = []
                for t in range(nt):
                    pu = pup.tile([128, smax], f32, tag="pu")
                    nc.tensor.matmul(
                        pu[:, :s_g],
                        c2t_t[:, (ntoffs[g] + t) * 128 : (ntoffs[g] + t + 1) * 128],
                        c1r_t[:, soffs[g] : soffs[g] + s_g],
                        start=True,
                        stop=True,
                    )
                    pus.append(pu)
                return pus

            for g, (s_g, nt, ns) in enumerate(slot_sizes):
                soff, ntoff, nsoff = soffs[g], ntoffs[g], nsoffs[g]
                pus = emit_dist(g)
                # 2) rec = 1/d2
                recs = []
                for t in range(nt):
                    rec = recp.tile([128, smax], f16, tag="rec")
                    recip_raw(rec[:, :s_g], pus[t][:, :s_g])
                    recs.append(rec)
                # 3) top8
                top8s = []
                for t in range(nt):
                    top8 = smallp.tile([128, 8], f16, tag="top8")
                    nc.vector.max(out=top8, in_=recs[t][:, :s_g])
                    top8s.append(top8)
                # 4) W = (rec >= th) * rec, sumw = rowsum(W)
                sumw = smallp.tile([128, ntmax], f32, tag="sumw")
                Ws = []
                for t in range(nt):
                    W = wp.tile([128, smax], f16, tag="W")
                    nc.vector.scalar_tensor_tensor(
                        out=W[:, :s_g],
                        in0=recs[t][:, :s_g],
                        scalar=top8s[t][:, 7:8],
                        in1=recs[t][:, :s_g],
                        op0=OP.is_ge,
                        op1=OP.mult,
                        accum_out=sumw[:, t : t + 1],
                    )
                    Ws.append(W)
                # 5) rsw = 1/sumw
                rsw = smallp.tile([128, ntmax], f32, tag="rsw")
                nc.vector.reciprocal(rsw[:, :nt], sumw[:, :nt])
                # 6) diag[t] = ident * rsw[:,t] -- all nt diagonals in ONE
                #    broadcast tensor_tensor: rsw bcast along j, ident along t
                dgc = diagp.tile([128, ntmax, 128], f16, tag="diag")
                nc.vector.tensor_tensor(
                    out=dgc[:, :nt],
                    in0=rsw[:, :nt].unsqueeze(2).broadcast_to([128, nt, 128]),
                    in1=ident.unsqueeze(1).broadcast_to([128, nt, 128]),
                    op=OP.mult,
                )
                diags = [dgc[:, t] for t in range(nt)]
                # 7) Wt[t] = W[t]^T @ diag[t]  (transpose + normalize fused)
                pwts = []
                for t in range(nt):
                    pwt = pwtp.tile([128, nsmax, 128], f32, tag="pwt")
                    for k in range(ns):
                        w0 = 128 * k
                        cw = min(s_g, w0 + 128) - w0
                        nc.tensor.matmul(
                            pwt[:cw, k],
                            Ws[t][:, w0 : w0 + cw],
                            diags[t],
                            start=True,
                            stop=True,
                        )
                    pwts.append(pwt)
                # 8) one batched PSUM->SBUF copy per target chunk
                wtss = []
                for t in range(nt):
                    wts = wtsp.tile([128, nsmax, 128], f16, tag="wts")
                    nc.scalar.copy(wts[:, :ns], pwts[t][:, :ns])
                    wtss.append(wts)
                # 9) interp[t] = sum_k wts[k]^T @ f1a[k]
                pi = pip_.tile([128, ntmax, D], f32, tag="pi")
                for t in range(nt):
                    for k in range(ns):
                        w0 = 128 * k
                        cw = min(s_g, w0 + 128) - w0
                        nc.tensor.matmul(
                            pi[:, t],
                            wtss[t][:cw, k],
                            f1a_t[:cw, nsoff + k],
                            start=(k == 0),
                            stop=(k == ns - 1),
                        )
                # 10) err = interp - f2  (one op per slot)
                err = errp.tile([128, ntmax, D], f16, tag="err")
                nc.vector.tensor_tensor(
                    out=err[:, :nt],
                    in0=pi[:, :nt],
                    in1=f2_t[:, ntoff : ntoff + nt],
                    op=OP.subtract,
                )
                # 11) acc[:, g] = sum(err^2) on ACT (DVE is the critical
                #     engine in steady state)
                sq = errp.tile([128, ntmax, D], f16, tag="sq")
                nc.scalar.activation(
                    sq[:, :nt],
                    err[:, :nt],
                    AF.Square,
                    accum_out=acc[:, g : g + 1],
                )

            tot = constp.tile([128, 1], f32)
            nc.vector.reduce_sum(tot, acc, axis=mybir.AxisListType.X)
            nc.sync.dma_start(out_d[:, :], tot)

    nc.compile()
    return nc


def _hl(x):
    """fp16 hi/lo split: x ~= hi + lo with both parts exact in fp16."""
    hi = x.astype(np.float16)
    lo = (x - hi.astype(np.float32)).astype(np.float16)
    return hi, lo


def _prep(inputs):
    """Returns (slot_sizes, in_maps): slot-shaped packing of the 64 graphs."""
    x1 = np.ascontiguousarray(np.asarray(inputs["x1"], dtype=np.float32))
    x2 = np.ascontiguousarray(np.asarray(inputs["x2"], dtype=np.float32))
    b1 = np.asarray(inputs["b1"]).astype(np.int64)
    b2 = np.asarray(inputs["b2"]).astype(np.int64)

    c1, f1 = x1[:, :3], x1[:, 3:]
    c2, f2 = x2[:, :3], x2[:, 3:]

    gs = np.arange(B + 1)
    e1 = np.searchsorted(b1, gs)
    e2 = np.searchsorted(b2, gs)
    n1 = np.diff(e1)
    n2 = np.diff(e2)
    assert n1.min() >= KNN, f"graph with fewer than {KNN} sources"
    assert n2.min() >= 1

    # Deal graphs into GPC rank-groups; group i -> slot i of every core.
    # Group by nt-class (ceil(n2/128)) so per-slot nt tracks each graph's
    # own chunk count, then WITHIN each nt-class group by n1 rank so that
    # per-slot source width (group max n1) is as tight as possible -- this
    # gives about half the slots ns=2 instead of all ns=3.
    order = np.argsort(-n2, kind="stable")
    nt_of = -(-n2 // 128)
    classes = {}
    for g in order:
        classes.setdefault(int(nt_of[g]), []).append(int(g))
    groups = []
    leftover = []
    for ntc in sorted(classes, reverse=True):
        glist = classes[ntc]
        glist = sorted(glist, key=lambda g: -int(n1[g]))
        while len(glist) >= NCORES:
            groups.append(np.array(glist[:NCORES]))
            glist = glist[NCORES:]
        leftover.extend(glist)
    while leftover:
        groups.append(np.array(leftover[:NCORES]))
        leftover = leftover[NCORES:]
    assert len(groups) == GPC and all(len(g) == NCORES for g in groups)

    slot_sizes = []
    for grp in groups:
        s = max(int(n1[grp].max()) + 1, KNN + 1)
        s = -(-s // 4) * 4
        nt = -(-int(n2[grp].max()) // 128)
        ns = -(-s // 128)
        slot_sizes.append((s, nt, ns))
    slot_sizes = tuple(slot_sizes)

    SS = sum(s for s, _, _ in slot_sizes)
    NTC = sum(nt for _, nt, _ in slot_sizes)
    NSC = sum(ns for _, _, ns in slot_sizes)

    in_maps = []
    for c in range(NCORES):
        c1r = np.zeros((KMM, SS), np.float16)
        c2t = np.zeros((KMM, NTC * 128), np.float16)
        f1a = np.zeros((128, NSC, D), np.float16)
        f2p = np.zeros((128, NTC, D), np.float16)

        soff = ntoff = nsoff = 0
        for gi in range(GPC):
            g = int(groups[gi][c])
            s_g, nt, ns = slot_sizes[gi]
            a, bnd = e1[g], e1[g + 1]
            n = int(n1[g])
            cc = np.full((s_g, 3), BIGC, np.float32)
            cc[:n] = c1[a:bnd]
            h1, l1 = _hl(cc)
            nrm1 = np.einsum("ij,ij->i", cc, cc)
            n1h, n1l = _hl(nrm1)
            blk = c1r[:, soff : soff + s_g]
            blk[0:3] = (2.0 * h1.astype(np.float32)).astype(np.float16).T
            blk[3:6] = (2.0 * l1.astype(np.float32)).astype(np.float16).T
            blk[6:9] = blk[0:3]
            blk[9] = -n1h
            blk[10] = -n1l
            blk[11:13] = 1.0

            ff = np.zeros((ns * 128, D), np.float32)
            ff[:n] = f1[a:bnd]
            f1a[:, nsoff : nsoff + ns] = (
                ff.reshape(ns, 128, D).transpose(1, 0, 2).astype(np.float16)
            )

            a2, bnd2 = e2[g], e2[g + 1]
            m = int(n2[g])
            tcd = np.full((nt * 128, 3), BIGC + 1.0, np.float32)
            tcd[:m] = c2[a2:bnd2]
            h2, l2 = _hl(tcd)
            nrm2 = np.einsum("ij,ij->i", tcd, tcd)
            n2h, n2l = _hl(nrm2)
            tblk = c2t[:, ntoff * 128 : (ntoff + nt) * 128]
            tblk[0:3] = h2.T
            tblk[3:6] = h2.T
            tblk[6:9] = l2.T
            tblk[9:11] = 1.0
            tblk[11] = -n2h
            tblk[12] = -n2l

            fv = np.zeros((nt * 128, D), np.float32)
            fv[:m] = f2[a2:bnd2]
            f2p[:, ntoff : ntoff + nt] = (
                fv.reshape(nt, 128, D).transpose(1, 0, 2).astype(np.float16)
            )

            soff += s_g
            ntoff += nt
            nsoff += ns

        in_maps.append(
            {
                "c1r": np.ascontiguousarray(c1r),
                "c2t": np.ascontiguousarray(c2t),
                "f1a": np.ascontiguousarray(f1a),
                "f2": np.ascontiguousarray(f2p),
            }
        )
    return slot_sizes, in_maps


def run(inputs, trace=False):
    """Returns (mse_scalar_f32, exec_time_ns_or_None)."""
    from concourse.bass_utils import run_bass_kernel_spmd

    slot_sizes, in_maps = _prep(inputs)
    nc = _NC_CACHE.get(slot_sizes)
    if nc is None:
        nc = _NC_CACHE.setdefault(slot_sizes, _build_nc(slot_sizes))
    res = run_bass_kernel_spmd(nc, in_maps, core_ids=list(range(NCORES)), trace=trace)
    total = 0.0
    for r in res.results:
        total += np.asarray(r["out_sums"], dtype=np.float64).sum()
    mse = np.float32(total / (N * D))
    return mse, res.exec_time_ns


def kernel(**inputs):
    out, _ = run(inputs, trace=False)
    return out
